# revision 24
# baseline (speedup 1.0000x reference)
"""AGCRN cell kernel for 8 Trainium2 NeuronCores.

Strategy: data-parallel over batch (B=32 -> 4 per core). Each core
redundantly builds S = exp(relu(E E^T)) (symmetric, so it serves directly
as the matmul stationary operand without any transpose) plus row sums d;
the adaptive-adjacency normalization 1/d is folded into PSUM evacuations
as a per-partition scale. The Chebyshev chain runs node-major; the
x_g @ W contraction transposes 96-column chunks on the PE (zero-padded W
rows absorb pad garbage, a ones-column provides the bias for free).
"""

import os
import sys

import numpy as np
import ml_dtypes

for _p in ("/opt/trn_rl_repo", "/root/.axon_site/_ro/trn_rl_repo"):
    if os.path.isdir(_p) and _p not in sys.path:
        sys.path.append(_p)

import concourse.bass as bass
import concourse.tile as tile
from concourse import bacc, mybir
from concourse.bass_utils import run_bass_kernel_spmd
from concourse.masks import make_identity

F32 = mybir.dt.float32
BF16 = mybir.dt.bfloat16
AF = mybir.ActivationFunctionType
ALU = mybir.AluOpType

P = 128          # partitions
N = 2048         # nodes
NT = N // P      # node tiles = 16
NB = 4           # batches per core
CH = 66          # dim_in + hidden
CPB = 96         # padded channel slot per batch (66 real + 1 ones + pad)
HID = 64
OC_G = 128       # gate output channels (2*hidden)
NCORES = 8
RT_GROUP = 16    # row-tiles per transpose/matmul group
TW = 68          # transposed-chunk rows (66 channels + ones column + pad)


def _dv(ap, nb=NB, w=CPB):
    """View a [P, nb*w] slice as [P, nb, w]."""
    return ap.rearrange("p (b c) -> p b c", b=nb)


def build_nc():
    nc = bacc.Bacc(
        "TRN2",
        target_bir_lowering=False,
        debug=False,
        enable_asserts=False,
        num_devices=NCORES,
    )
    x_d = nc.dram_tensor("x", [NB, N, 2], F32, kind="ExternalInput").ap()
    st_d = nc.dram_tensor("state", [NB, N, HID], F32, kind="ExternalInput").ap()
    # [E_hi; E_lo; E_hi] — with rhs [E_hi; E_hi; E_lo] gives
    # Eh·Eh + El·Eh + Eh·El ≈ E·E in full f32 precision (bf16 products are
    # exact in f32 accumulation; the dropped El·El term is ~1e-5 relative).
    et_d = nc.dram_tensor("et", [2, 30, N], BF16, kind="ExternalInput").ap()
    wg_d = nc.dram_tensor("wg", [3, TW, OC_G], BF16, kind="ExternalInput").ap()
    wu_d = nc.dram_tensor("wu", [3, TW, HID], BF16, kind="ExternalInput").ap()
    out_d = nc.dram_tensor("out", [NB, N, HID], F32, kind="ExternalOutput").ap()

    with tile.TileContext(nc) as tc:
        _build(tc, x_d, st_d, et_d, wg_d, wu_d, out_d)
    nc.compile()
    return nc


def _build(tc, x_d, st_d, et_d, wg_d, wu_d, out_d):
    nc = tc.nc
    from contextlib import ExitStack

    with ExitStack() as ctx:
        const = ctx.enter_context(tc.tile_pool(name="const", bufs=1))
        persist = ctx.enter_context(tc.tile_pool(name="persist", bufs=1))

        ident = const.tile([P, P], BF16)
        make_identity(nc, ident)

        # ~3.5us of dummy matmuls: pushes the PE HAM clock-gate to 8/8
        # before the real work arrives
        with tc.tile_pool(name="warm", bufs=1, space="PSUM") as warm:
            wp = warm.tile([P, P], F32)
            for _ in range(32):
                nc.tensor.matmul(wp[:], lhsT=ident[:], rhs=ident[:], start=True, stop=True)

        etp = const.tile([30, 2, N], BF16)
        nc.scalar.dma_start(etp[:, 0, :], et_d[0])
        nc.scalar.dma_start(etp[:, 1, :], et_d[1])
        wg_sb = const.tile([TW, 3, OC_G], BF16)
        wu_sb = const.tile([TW, 3, HID], BF16)
        for k in range(3):
            nc.scalar.dma_start(wg_sb[:, k, :], wg_d[k])
            nc.scalar.dma_start(wu_sb[:, k, :], wu_d[k])

        S_sb = persist.tile([P, NT, N], BF16)       # S row-tiles
        x0_sb = persist.tile([P, NT, NB * CPB], BF16)
        u1_sb = persist.tile([P, NT, NB * CPB], BF16)
        u2_sb = persist.tile([P, NT, NB * CPB], BF16)
        stt_sb = persist.tile([P, NT, NB, HID], BF16)   # state copy for epilogue
        zr_sb = persist.tile([P, NT, NB, OC_G], BF16)   # sigmoid(gate)
        dtot = persist.tile([P, NT], F32)
        rinv = persist.tile([P, NT], F32)
        rinv2 = persist.tile([P, NT], F32)

        nc.gpsimd.memset(x0_sb[:], 0.0)
        nc.gpsimd.memset(u1_sb[:], 0.0)
        nc.gpsimd.memset(u2_sb[:], 0.0)
        for b in range(NB):
            # ones column feeding the bias row of W chunk 2
            nc.gpsimd.memset(u2_sb[:, :, b * CPB + CH : b * CPB + CH + 1], 1.0)

        # ---- input load (one big strided DMA per batch) + convert ----
        inp_pool = ctx.enter_context(tc.tile_pool(name="inp", bufs=1))
        for b in range(NB):
            stf = inp_pool.tile([P, NT, HID], F32, tag=f"stf{b}")
            xf = inp_pool.tile([P, NT, 2], F32, tag=f"xf{b}")
            nc.sync.dma_start(stf[:], st_d[b].rearrange("(t p) h -> p t h", p=P))
            nc.sync.dma_start(xf[:], x_d[b].rearrange("(t p) h -> p t h", p=P))
            # x0 state cols for batch b across all node tiles
            nc.vector.tensor_copy(
                x0_sb[:, :, b * CPB + 2 : b * CPB + 2 + HID], stf[:]
            )
            nc.vector.tensor_copy(x0_sb[:, :, b * CPB : b * CPB + 2], xf[:])
            nc.vector.tensor_copy(stt_sb[:, :, b, :], stf[:])

        # ---- S = exp(relu(E E^T)) with row sums, fused with the first
        # WAVE mt-groups of gconv1's first chain application (their
        # matmuls consume S row-tiles as they are produced, keeping the
        # PE busy while DVE-relu/ACT-exp pipeline through) ----
        cpsum = ctx.enter_context(tc.tile_pool(name="cpsum", bufs=1, space="PSUM"))
        WAVE = 4
        wave_cp = {}
        for mt in range(WAVE):
            wave_cp[mt] = cpsum.tile([P, NB * CH], F32, tag=f"wv{mt}", name=f"wavecp{mt}")
        with (
            tc.tile_pool(name="lpsum", bufs=2, space="PSUM") as lpsum,
            tc.tile_pool(name="lrelu", bufs=3) as lrelu,
        ):
            for kt in range(NT):
                lr = lrelu.tile([P, N], F32)
                for q in range(2):
                    lp = lpsum.tile([P, 1024], F32)
                    for h in range(2):
                        nc.tensor.matmul(
                            lp[:, h * 512 : (h + 1) * 512],
                            lhsT=etp[:, 0, kt * P : (kt + 1) * P],
                            rhs=etp[:, 1, q * 1024 + h * 512 : q * 1024 + (h + 1) * 512],
                            start=True,
                            stop=True,
                        )
                    nc.vector.tensor_scalar_max(
                        lr[:, q * 1024 : (q + 1) * 1024], lp[:], 0.0
                    )
                # exp + row-sum in one big ACT op
                nc.scalar.activation(
                    S_sb[:, kt, :], lr[:], AF.Exp,
                    accum_out=dtot[:, kt : kt + 1],
                )
                nc.vector.reciprocal(rinv[:, kt : kt + 1], dtot[:, kt : kt + 1])
                nc.vector.tensor_scalar_mul(
                    rinv2[:, kt : kt + 1], rinv[:, kt : kt + 1], 2.0
                )
                # chain wave: one k-step for the first WAVE output groups
                for mt in range(WAVE):
                    nc.tensor.matmul(
                        wave_cp[mt][:],
                        lhsT=S_sb[:, kt, mt * P : (mt + 1) * P],
                        rhs=_dv(x0_sb[:, kt, :])[:, :, 0:CH],
                        start=(kt == 0),
                        stop=(kt == NT - 1),
                    )
        for mt in range(WAVE):
            nc.scalar.activation(
                _dv(u1_sb[:, mt, :])[:, :, 0:CH], wave_cp[mt][:],
                AF.Copy, scale=rinv[:, mt : mt + 1],
            )

        tpsum = ctx.enter_context(tc.tile_pool(name="tpsum", bufs=2, space="PSUM"))
        zpsum = ctx.enter_context(tc.tile_pool(name="zpsum", bufs=2, space="PSUM"))
        xgt_pool = ctx.enter_context(tc.tile_pool(name="xgt", bufs=3 * RT_GROUP))
        epi_pool = ctx.enter_context(tc.tile_pool(name="epi", bufs=6))

        def apply_S(src, dst, second, mt0=0):
            """dst = (S @ src) / d   (or 2*(S @ src)/d - x0 when second)."""
            for mt in range(mt0, NT):
                cp = cpsum.tile([P, NB * CH], F32, tag=f"wv{mt % 4}", name=f"cp{mt}")
                for kt in range(NT):
                    nc.tensor.matmul(
                        cp[:],
                        lhsT=S_sb[:, kt, mt * P : (mt + 1) * P],
                        rhs=_dv(src[:, kt, :])[:, :, 0:CH],
                        start=(kt == 0),
                        stop=(kt == NT - 1),
                    )
                dstv = _dv(dst[:, mt, :])[:, :, 0:CH]
                if not second:
                    nc.scalar.activation(
                        dstv, cp[:], AF.Copy, scale=rinv[:, mt : mt + 1]
                    )
                else:
                    nc.vector.scalar_tensor_tensor(
                        out=dstv,
                        in0=cp[:],
                        scalar=rinv2[:, mt : mt + 1],
                        in1=_dv(x0_sb[:, mt, :])[:, :, 0:CH],
                        op0=ALU.mult,
                        op1=ALU.subtract,
                    )

        def gconv_tail(gate):
            """Transpose x_g chunks + W matmul + nonlinearity (+ epilogue).

            Work is batched per node-tile: the 4 batches' W-matmul outputs
            share one PSUM bank so sigmoid/tanh/epilogue run as single wide
            ops, and output DMA is one strided transfer per node tile.
            """
            w_sb = wg_sb if gate else wu_sb
            oc = OC_G if gate else HID
            for nt0 in range(0, NT, RT_GROUP // NB):
                nts = range(nt0, nt0 + RT_GROUP // NB)
                xgts = {}
                for nt in nts:
                    for b in range(NB):
                        tp = tpsum.tile([TW, 3, P], BF16)
                        for k, srcb in enumerate((x0_sb, u1_sb, u2_sb)):
                            nc.tensor.transpose(
                                tp[:, k, :],
                                srcb[:, nt, b * CPB : b * CPB + TW],
                                ident[:],
                            )
                        xgt = xgt_pool.tile([TW, 3, P], BF16)
                        # balance PSUM evacuations across DVE and ACT
                        if b % 2 == 0:
                            nc.vector.tensor_copy(xgt[:], tp[:])
                        else:
                            nc.scalar.copy(xgt[:], tp[:])
                        xgts[(nt, b)] = xgt
                for nt in nts:
                    zp = zpsum.tile([P, NB, oc], F32, tag="zp")
                    for b in range(NB):
                        for k in range(3):
                            nc.tensor.matmul(
                                zp[:, b, :],
                                lhsT=xgts[(nt, b)][:, k, :],
                                rhs=w_sb[:, k, :],
                                start=(k == 0),
                                stop=(k == 2),
                            )
                    if gate:
                        nc.scalar.activation(zr_sb[:, nt], zp[:], AF.Sigmoid)
                        # candidate: state-cols of x0 *= z  (in place, all b)
                        x0c = _dv(x0_sb[:, nt, :])[:, :, 2 : 2 + HID]
                        nc.vector.tensor_mul(
                            x0c, x0c, zr_sb[:, nt, :, 0:HID]
                        )
                    else:
                        hc = epi_pool.tile([P, NB, HID], BF16, tag="hc")
                        nc.scalar.activation(hc[:], zp[:], AF.Tanh)
                        r = zr_sb[:, nt, :, HID:OC_G]
                        t1 = epi_pool.tile([P, NB, HID], BF16, tag="t1")
                        nc.vector.tensor_sub(t1[:], stt_sb[:, nt], hc[:])
                        hf = epi_pool.tile([P, NB, HID], F32, tag="hf")
                        # h = hc + r*(state - hc)
                        nc.vector.scalar_tensor_tensor(
                            out=hf[:],
                            in0=t1[:],
                            scalar=1.0,
                            in1=r,
                            op0=ALU.mult,
                            op1=ALU.mult,
                        )
                        nc.vector.tensor_add(hf[:], hf[:], hc[:])
                        nc.sync.dma_start(
                            out_d[:, nt * P : (nt + 1) * P, :].rearrange(
                                "b p h -> p b h"
                            ),
                            hf[:],
                        )

        # gconv 1 (gate) -- first WAVE groups of app1 were fused above
        apply_S(x0_sb, u1_sb, second=False, mt0=WAVE)
        apply_S(u1_sb, u2_sb, second=True)
        gconv_tail(gate=True)
        # gconv 2 (update) -- x0_sb now holds the candidate input
        apply_S(x0_sb, u1_sb, second=False)
        apply_S(u1_sb, u2_sb, second=True)
        gconv_tail(gate=False)


_NC = None


def _get_nc():
    global _NC
    if _NC is None:
        _NC = build_nc()
    return _NC


def _prep_in_maps(x, state, node_embeddings, W_gate, b_gate, W_update, b_update):
    bf = ml_dtypes.bfloat16
    x = np.asarray(x, dtype=np.float32)
    state = np.asarray(state, dtype=np.float32)
    E = np.asarray(node_embeddings, dtype=np.float32)
    W_gate = np.asarray(W_gate, dtype=np.float32)
    b_gate = np.asarray(b_gate, dtype=np.float32)
    W_update = np.asarray(W_update, dtype=np.float32)
    b_update = np.asarray(b_update, dtype=np.float32)

    eh = E.T.astype(bf)                       # [10, N] bf16
    el = (E.T - eh.astype(np.float32)).astype(bf)
    stack_l = np.concatenate([eh, el, eh], axis=0)   # lhsT rows
    stack_r = np.concatenate([eh, eh, el], axis=0)   # rhs rows
    et = np.ascontiguousarray(np.stack([stack_l, stack_r]))  # [2, 30, N]
    wg = np.zeros((3, 68, OC_G), np.float32)
    wu = np.zeros((3, 68, HID), np.float32)
    for k in range(3):
        wg[k, :CH] = W_gate[CH * k : CH * (k + 1)]
        wu[k, :CH] = W_update[CH * k : CH * (k + 1)]
    wg[2, CH] = b_gate
    wu[2, CH] = b_update
    wg = wg.astype(bf)
    wu = wu.astype(bf)

    in_maps = []
    for r in range(NCORES):
        in_maps.append(
            {
                "x": np.ascontiguousarray(x[NB * r : NB * (r + 1)]),
                "state": np.ascontiguousarray(state[NB * r : NB * (r + 1)]),
                "et": et,
                "wg": wg,
                "wu": wu,
            }
        )
    return in_maps


def run(trace=False, **inputs):
    nc = _get_nc()
    in_maps = _prep_in_maps(**inputs)
    res = run_bass_kernel_spmd(
        nc, in_maps, core_ids=list(range(NCORES)), trace=trace
    )
    out = np.concatenate([res.results[r]["out"] for r in range(NCORES)], axis=0)
    return out, res


def kernel(**inputs) -> np.ndarray:
    out, _ = run(trace=False, **inputs)
    return out
